# revision 7
# baseline (speedup 1.0000x reference)
"""Trainium2 Bass kernel for BilinearInteraction.

Reference math (B=2048, F=32 fields, D=64, P=496 field-pairs):
    for pair p=(i,j):  out[b,p,:] = (v_i @ W[p].T) * v_j
    v_i = feature_emb[:, i, :],  v_j = feature_emb[:, j, :]

Sharding: data-parallel over batch, 8 cores x 256 rows each; W replicated.
The fp32 output is 260MB (32.5MB/core); the device computes in bf16
(out 16.25MB/core) and the host upcasts; rel err ~3e-3 vs the 2e-2 gate.

Per-core design, driven by measured engine rails (PE matmul streams at
its 1.2GHz p-state ~57us for the 63.5K output columns; DVE/ACT evacuate
PSUM at ~105 elem/ns each; GpSimd multiplies at ~55; one HWDGE queue
sustains only ~283 GB/s - the inter-DMA completion handshake idles the
engines - so the 22.6MB of traffic needs all three DMA queues):
  - Resident SBUF tiles: w[4] [128,4096] bf16 (pre-transposed W: partition
    half 0 = pairs 0..255 with col p*64+e = W[p,e,d=partition], half 1 =
    pairs 256..495); ft [128, 5632] bf16 (per-field transposed features,
    matmul lhsT: partitions 0:64 = fields 0..9 at col f*256+bc*128+b,
    64:128 = fields 9..30); fn[2] [128, 2048] bf16 (natural-layout
    features, the Hadamard multiplier; field 0 is never a second field).
  - Every input DMA is one full contiguous DRAM tensor (a column-slice
    of a row-major DRAM tensor emits 128 tiny strided descriptors and
    runs descriptor-bound). Chunks are spread over the SP ring (prologue:
    ft/fn head), ACT ring (W blocks) and the gpsimd SWDGE ring (late ft,
    fn1) in first-use order, so the first matmul issues ~10us in and no
    ring serializes compute.
  - Compute: per (batch-half bc, stage of 8..48 pairs): pairs grouped
    into runs (same first field, one W block, <=16 pairs). Each run:
    1-2 matmuls [K=64,M=128]x[N<=512] bf16 (the ISA caps one matmul at
    512 output elements = one PSUM bank) into a 2-bank PSUM tile
    (bufs=4 -> ~3.4us of PE runway against drain latency), then the
    Hadamard via one of three engine paths balanced to equalize busy
    time at measured rates:
      A: DVE   tensor_mul(psum_f32, fnb_bf16)            -> stage bf16
      B: ACT   copy psum -> tmp bf16;  DVE mul(tmp, fnb) -> stage bf16
      C: ACT   copy psum -> tmp bf16;  GPS mul(tmp, fnb) -> stage bf16
  - Each stage's bf16 block goes out as one DMA to its own contiguous
    DRAM tensor o{bc}_{si}; the host scatters the blocks into the full
    [B, P, D] f32 output. Output stages are spread over the three queues
    ~50/30/20; DMAs issued from the ACT and GPS queues are emitted two
    stages late so their stage semaphore is already fired when those
    sequencers reach them (an eager emission would head-of-line-block
    the copies / muls queued behind the wait).
"""

from itertools import combinations

import numpy as np

N_CORES = 8
B, F, D = 2048, 32, 64
P = 496
B_SH = B // N_CORES            # 256 batch rows per core
HALF = 256                     # pair index where the partition half flips
RUN = 16                       # max pairs per run (one 2-bank PSUM tile)

# Hadamard path element-share targets (measured-rate LP)
SHARE = {"A": 0.386, "B": 0.317, "C": 0.297}

# output stages; bc=0 starts small to prime the output stream, bc=1 ends
# small to shorten the serial kernel tail
_SIZES0 = [8, 8, 16, 16, 32, 32] + [48] * 8
_SIZES1 = [48] * 8 + [32, 32, 16, 16, 8, 8]


def _bounds(sizes):
    b, acc = [0], 0
    for s in sizes:
        acc += s
        b.append(acc)
    assert acc == P
    return list(zip(b[:-1], b[1:]))


STAGES = {0: _bounds(_SIZES0), 1: _bounds(_SIZES1)}


def _out_ring(bc, si):
    if bc == 0 and si < 3:
        return "sp"              # prime the pipe with the shortest chain
    if bc == 1 and si >= 12:
        return "sp"              # tail stages: immediate, unlagged
    g = bc * len(STAGES[0]) + si
    if g % 2 == 0:
        return "sp"
    return "act" if g % 4 == 1 else "gps"


# input chunks: (name, ring, dest, part_hi, col_lo, col_hi), in first-use
# order per ring.
IN_CHUNKS = [
    ("ft_a", "sp", "ft", 128, 0, 512),
    ("fn0_a", "sp", "fn0", 128, 64, 1152),
    ("ft_b", "sp", "ft", 128, 512, 1024),
    ("fn0_b", "sp", "fn0", 128, 1152, 2048),
    ("w0_a", "act", "w0", 128, 0, 1024),
    ("w0_b", "act", "w0", 128, 1024, 4096),
    ("w1", "act", "w1", 128, 0, 4096),
    ("w2", "act", "w2", 128, 0, 4096),
    ("w3_a", "act", "w3", 128, 0, 3072),
    ("w3_b", "gps", "w3", 64, 3072, 4096),
    ("ft_c", "gps", "ft", 128, 1024, 2560),
    ("ft_d", "gps", "ft", 128, 2560, 5632),
    ("fn1", "gps", "fn1", 128, 64, 2048),
]

PAIRS = list(combinations(range(F), 2))

_NC_CACHE = {}


def _runs(lo, hi):
    """Runs of consecutive same-group pairs (<=RUN) in [lo,hi), not
    crossing 64-pair W-block boundaries."""
    runs = []
    p = lo
    while p < hi:
        i = PAIRS[p][0]
        e = p
        while (e + 1 < hi and PAIRS[e + 1][0] == i and (e + 1 - p) < RUN
               and (e + 1) % 64 != 0):
            e += 1
        runs.append((p, e - p + 1))
        p = e + 1
    return runs


def _build():
    import concourse.tile as tile
    from concourse import bacc, mybir

    F32 = mybir.dt.float32
    BF16 = mybir.dt.bfloat16
    nc = bacc.Bacc("TRN2", target_bir_lowering=False, debug=False,
                   enable_asserts=False, num_devices=N_CORES)

    chunks = {}
    for name, ring, dest, ph, c0, c1 in IN_CHUNKS:
        chunks[name] = nc.dram_tensor(name, [ph, c1 - c0], BF16,
                                      kind="ExternalInput").ap()
    outs = {}
    for bc in range(2):
        for si, (lo, hi) in enumerate(STAGES[bc]):
            outs[(bc, si)] = nc.dram_tensor(
                f"o{bc}_{si}", [128, (hi - lo) * D], BF16,
                kind="ExternalOutput").ap()

    with tile.TileContext(nc) as tc:
        with (
            tc.tile_pool(name="win", bufs=1) as win,
            tc.tile_pool(name="feat", bufs=1) as feat,
            tc.tile_pool(name="stage", bufs=8) as stage_pool,
            tc.tile_pool(name="tmp", bufs=8) as tmp_pool,
            tc.tile_pool(name="psum", bufs=4, space="PSUM") as psum_pool,
        ):
            # resident input tiles ------------------------------------------------
            w = [win.tile([128, 4096], BF16, name=f"w{blk}", tag=f"w{blk}")
                 for blk in range(4)]
            ft = feat.tile([128, 22 * B_SH], BF16, name="ft", tag="ft")
            fn = [feat.tile([128, F * D], BF16, name=f"fn{bc}", tag=f"fn{bc}")
                  for bc in range(2)]
            tiles = {"ft": ft, "fn0": fn[0], "fn1": fn[1],
                     **{f"w{i}": w[i] for i in range(4)}}
            engs = {"sp": nc.sync, "act": nc.scalar, "gps": nc.gpsimd}

            for name, ring, dest, ph, c0, c1 in IN_CHUNKS:
                engs[ring].dma_start(tiles[dest][0:ph, c0:c1],
                                     chunks[name][:, :])

            # compute + output ----------------------------------------------------
            done = {"A": 0, "B": 0, "C": 0}

            def pick(n):
                tot = sum(done.values()) + n
                return max("ABC", key=lambda p: SHARE[p] * tot - done[p])

            pend = {"act": [], "gps": []}   # lagged output DMAs

            for bc in range(2):
                for si, (lo, hi) in enumerate(STAGES[bc]):
                    st = stage_pool.tile([128, (hi - lo) * D], BF16, tag="stage")
                    for (p0, n) in _runs(lo, hi):
                        i, j0 = PAIRS[p0]
                        h = p0 // HALF
                        po = 64 * h
                        fcol = (i - 9 * h) * B_SH   # field col in ft's half
                        colbase = (p0 - h * HALF) * D
                        blk, bcol = colbase // 4096, colbase % 4096
                        ps = psum_pool.tile([128, RUN * D], F32, tag="ps")
                        for k in range(0, n, 8):
                            nk = min(8, n - k)
                            nc.tensor.matmul(
                                ps[:, k * D:(k + nk) * D],
                                lhsT=ft[po:po + 64,
                                        fcol + bc * 128:
                                        fcol + bc * 128 + 128],
                                rhs=w[blk][po:po + 64,
                                           bcol + k * D: bcol + (k + nk) * D],
                                start=True, stop=True,
                            )
                        st_sl = st[:, (p0 - lo) * D: (p0 - lo + n) * D]
                        fn_sl = fn[bc][:, j0 * D: (j0 + n) * D]
                        if (bc == 0 and si < 3) or (bc == 1 and si >= 12):
                            path = "A"      # shortest chain at head and tail
                        else:
                            path = pick(n)
                        done[path] += n
                        if path == "A":
                            nc.vector.tensor_mul(st_sl, ps[:, 0:n * D], fn_sl)
                        else:
                            tmp = tmp_pool.tile([128, RUN * D], BF16, tag="tmp")
                            nc.scalar.copy(tmp[:, 0:n * D], ps[:, 0:n * D])
                            if path == "B":
                                nc.vector.tensor_mul(st_sl, tmp[:, 0:n * D],
                                                     fn_sl)
                            else:
                                nc.gpsimd.tensor_mul(st_sl, tmp[:, 0:n * D],
                                                     fn_sl)
                    ring = _out_ring(bc, si)
                    if ring == "sp":
                        nc.sync.dma_start(outs[(bc, si)][:, :], st[:, :])
                    else:
                        pend[ring].append((outs[(bc, si)][:, :], st[:, :]))
                    for r in ("act", "gps"):
                        if len(pend[r]) > 1:
                            dst, src = pend[r].pop(0)
                            engs[r].dma_start(dst, src)
            for r in ("act", "gps"):
                for dst, src in pend[r]:
                    engs[r].dma_start(dst, src)
    nc.compile()
    return nc


def _pack_inputs(feature_emb, W):
    import ml_dtypes

    BF = ml_dtypes.bfloat16
    feature_emb = np.ascontiguousarray(feature_emb, dtype=np.float32)
    W = np.ascontiguousarray(W, dtype=np.float32)
    Wt = W.transpose(0, 2, 1)                      # [P, d, e]
    wpack = np.zeros((128, 4 * 4096), dtype=BF)
    wpack[0:64, :] = Wt[0:HALF].transpose(1, 0, 2).reshape(64, HALF * D).astype(BF)
    wpack[64:128, 0:(P - HALF) * D] = (
        Wt[HALF:P].transpose(1, 0, 2).reshape(64, (P - HALF) * D).astype(BF))
    in_maps = []
    for c in range(N_CORES):
        shard = feature_emb[c * B_SH:(c + 1) * B_SH]         # [256, 32, 64]
        # [d, f, b] per-field transposed features
        ftT = shard.transpose(2, 1, 0).astype(BF)            # [64, 32, 256]
        featT = np.zeros((128, 22 * B_SH), dtype=BF)
        # partitions 0:64 <- fields 0..9 (first-fields of pairs 0..255)
        featT[0:64, 0:10 * B_SH] = ftT[:, 0:10].reshape(64, 10 * B_SH)
        # partitions 64:128 <- fields 9..30 (first-fields of pairs 256..495)
        featT[64:128, :] = ftT[:, 9:31].reshape(64, 22 * B_SH)
        fnb = shard.reshape(B_SH, F * D).astype(BF)          # [256, 2048]
        srcs = {"ft": featT, "fn0": fnb[0:128], "fn1": fnb[128:256],
                **{f"w{i}": wpack[:, i * 4096:(i + 1) * 4096]
                   for i in range(4)}}
        in_maps.append({
            name: np.ascontiguousarray(srcs[dest][0:ph, c0:c1])
            for name, ring, dest, ph, c0, c1 in IN_CHUNKS
        })
    return in_maps


def kernel(feature_emb, W, _trace=False):
    from concourse.bass_utils import run_bass_kernel_spmd

    if "nc" not in _NC_CACHE:
        _NC_CACHE["nc"] = _build()
    nc = _NC_CACHE["nc"]
    in_maps = _pack_inputs(feature_emb, W)
    res = run_bass_kernel_spmd(nc, in_maps, core_ids=list(range(N_CORES)),
                               trace=_trace)
    out = np.empty((B, P * D), dtype=np.float32)
    for c in range(N_CORES):
        r = res.results[c]
        for bc in range(2):
            rows = slice(c * B_SH + bc * 128, c * B_SH + bc * 128 + 128)
            for si, (lo, hi) in enumerate(STAGES[bc]):
                out[rows, lo * D:hi * D] = r[f"o{bc}_{si}"].astype(np.float32)
    out = out.reshape(B, P, D)
    if _trace:
        return out, res
    return out


# revision 8
# speedup vs baseline: 1.0006x; 1.0006x over previous
"""Trainium2 Bass kernel for BilinearInteraction.

Reference math (B=2048, F=32 fields, D=64, P=496 field-pairs):
    for pair p=(i,j):  out[b,p,:] = (v_i @ W[p].T) * v_j
    v_i = feature_emb[:, i, :],  v_j = feature_emb[:, j, :]

Sharding: data-parallel over batch, 8 cores x 256 rows each; W replicated.
The fp32 output is 260MB (32.5MB/core); the device computes in bf16
(out 16.25MB/core) and the host upcasts; rel err ~3e-3 vs the 2e-2 gate.

Per-core design, driven by measured engine rails (PE matmul streams its
63.5K output columns at the 1.2GHz p-state ~54us; DVE/ACT evacuate PSUM
at ~105 elem/ns each; GpSimd multiplies at ~55; one HWDGE queue sustains
only ~283 GB/s because the per-DMA completion handshake idles the
engines, so outputs are spread over both HWDGE queues and stages are
large to amortize handshakes):
  - Resident SBUF tiles: w[4] [128,4096] bf16 (pre-transposed W: partition
    half 0 = pairs 0..255 with col p*64+e = W[p,e,d=partition], half 1 =
    pairs 256..495); ft [128, 5632] bf16 (per-field transposed features,
    matmul lhsT: partitions 0:64 = fields 0..9 at col f*256+bc*128+b,
    64:128 = fields 9..30); fn[2] [128, 2048] bf16 (natural-layout
    features, the Hadamard multiplier; field 0 is never a second field).
  - Every input DMA is one full contiguous DRAM tensor (a column-slice
    of a row-major DRAM tensor emits 128 tiny strided descriptors and
    runs descriptor-bound). SP ring: small ft/fn prologue in first-use
    order, then output stages. ACT ring: the W blocks (wait-free; they
    are issued before the ACT copies start). GPS/SWDGE ring: the two
    late inputs (ft tail, fn1) so they cost no HWDGE queue time.
  - A short burst of dummy warm-up matmuls runs in the shadow of the
    input load to ramp the PE p-state before the real stream starts.
  - Compute: per (batch-half bc, stage of 8..64 pairs): pairs grouped
    into runs (same first field, one W block, <=16 pairs). Each run:
    1-2 matmuls [K=64,M=128]x[N<=512] bf16 (the ISA caps one matmul at
    512 output elements = one PSUM bank) into PSUM, then the Hadamard
    via one of three engine paths balanced to equalize busy time at
    measured rates:
      A: DVE   tensor_mul(psum_f32, fnb_bf16)            -> stage bf16
      B: ACT   copy psum -> tmp bf16;  DVE mul(tmp, fnb) -> stage bf16
      C: ACT   copy psum -> tmp bf16;  GPS mul(tmp, fnb) -> stage bf16
    Head and tail stages are forced to path A: at the head the ACT
    sequencer is still issuing input DMAs, at the tail the A chain is
    the shortest.
  - Each stage's bf16 block goes out as one DMA to its own contiguous
    DRAM tensor o{bc}_{si}; the host scatters the blocks into the full
    [B, P, D] f32 output. Stages alternate SP ring (immediate) / ACT
    ring; ACT-ring output DMAs are emitted two stages late so their
    stage semaphore is already fired when the ACT sequencer reaches
    them (an eager emission would head-of-line-block the ACT copies
    behind the wait).
"""

from itertools import combinations

import numpy as np

N_CORES = 8
B, F, D = 2048, 32, 64
P = 496
B_SH = B // N_CORES            # 256 batch rows per core
HALF = 256                     # pair index where the partition half flips
RUN = 16                       # max pairs per run (one 2-bank PSUM tile)
N_WARM = 8                     # dummy warm-up matmuls for the PE p-state

# Hadamard path element-share targets (measured-rate LP)
SHARE = {"A": 0.37, "B": 0.30, "C": 0.33}

# output stages; bc=0 starts small to prime the output stream, bc=1 ends
# small to shorten the serial kernel tail
_SIZES0 = [8, 8, 16, 16, 32, 48] + [64] * 5 + [48]
_SIZES1 = [48] + [64] * 5 + [48, 32, 16, 16, 8, 8]


def _bounds(sizes):
    b, acc = [0], 0
    for s in sizes:
        acc += s
        b.append(acc)
    assert acc == P
    return list(zip(b[:-1], b[1:]))


STAGES = {0: _bounds(_SIZES0), 1: _bounds(_SIZES1)}

# stages whose Hadamard path is forced to A (head: ACT busy issuing
# inputs; tail: shortest chain)
FORCE_A = {(0, 0), (0, 1), (0, 2), (0, 3), (1, 10), (1, 11)}


def _out_ring(bc, si):
    if bc == 0 and si < 4:
        return "sp"              # prime the pipe
    if bc == 1 and si >= 10:
        return "sp"              # tail stages: immediate, unlagged
    g = bc * len(STAGES[0]) + si
    return "sp" if g % 2 == 0 else "act"


# input chunks: (name, ring, dest, part_hi, col_lo, col_hi), in first-use
# order per ring.
IN_CHUNKS = [
    ("ft_a", "sp", "ft", 128, 0, 512),
    ("fn0_a", "sp", "fn0", 128, 64, 1152),
    ("ft_b", "sp", "ft", 128, 512, 1024),
    ("fn0_b", "sp", "fn0", 128, 1152, 2048),
    ("ft_c", "sp", "ft", 128, 1024, 2560),
    ("w0_a", "act", "w0", 128, 0, 1024),
    ("w0_b", "act", "w0", 128, 1024, 4096),
    ("w1", "act", "w1", 128, 0, 4096),
    ("w2", "act", "w2", 128, 0, 4096),
    ("w3", "act", "w3", 128, 0, 4096),
    ("ft_d", "gps", "ft", 128, 2560, 5632),
    ("fn1", "gps", "fn1", 128, 64, 2048),
]

PAIRS = list(combinations(range(F), 2))

_NC_CACHE = {}


def _runs(lo, hi):
    """Runs of consecutive same-group pairs (<=RUN) in [lo,hi), not
    crossing 64-pair W-block boundaries."""
    runs = []
    p = lo
    while p < hi:
        i = PAIRS[p][0]
        e = p
        while (e + 1 < hi and PAIRS[e + 1][0] == i and (e + 1 - p) < RUN
               and (e + 1) % 64 != 0):
            e += 1
        runs.append((p, e - p + 1))
        p = e + 1
    return runs


def _build():
    import concourse.tile as tile
    from concourse import bacc, mybir

    F32 = mybir.dt.float32
    BF16 = mybir.dt.bfloat16
    nc = bacc.Bacc("TRN2", target_bir_lowering=False, debug=False,
                   enable_asserts=False, num_devices=N_CORES)

    chunks = {}
    for name, ring, dest, ph, c0, c1 in IN_CHUNKS:
        chunks[name] = nc.dram_tensor(name, [ph, c1 - c0], BF16,
                                      kind="ExternalInput").ap()
    outs = {}
    for bc in range(2):
        for si, (lo, hi) in enumerate(STAGES[bc]):
            outs[(bc, si)] = nc.dram_tensor(
                f"o{bc}_{si}", [128, (hi - lo) * D], BF16,
                kind="ExternalOutput").ap()

    with tile.TileContext(nc) as tc:
        with (
            tc.tile_pool(name="win", bufs=1) as win,
            tc.tile_pool(name="feat", bufs=1) as feat,
            tc.tile_pool(name="stage", bufs=6) as stage_pool,
            tc.tile_pool(name="tmp", bufs=8) as tmp_pool,
            tc.tile_pool(name="psum", bufs=4, space="PSUM") as psum_pool,
        ):
            # resident input tiles ------------------------------------------------
            w = [win.tile([128, 4096], BF16, name=f"w{blk}", tag=f"w{blk}")
                 for blk in range(4)]
            ft = feat.tile([128, 22 * B_SH], BF16, name="ft", tag="ft")
            fn = [feat.tile([128, F * D], BF16, name=f"fn{bc}", tag=f"fn{bc}")
                  for bc in range(2)]
            warm = feat.tile([128, 512], BF16, name="warm", tag="warm")
            tiles = {"ft": ft, "fn0": fn[0], "fn1": fn[1],
                     **{f"w{i}": w[i] for i in range(4)}}
            engs = {"sp": nc.sync, "act": nc.scalar, "gps": nc.gpsimd}

            for name, ring, dest, ph, c0, c1 in IN_CHUNKS:
                engs[ring].dma_start(tiles[dest][0:ph, c0:c1],
                                     chunks[name][:, :])

            # PE p-state warm-up in the input-load shadow: results unused
            nc.vector.memset(warm[:, :], 0)
            for _ in range(N_WARM):
                psw = psum_pool.tile([128, RUN * D], F32, tag="ps", bufs=3)
                nc.tensor.matmul(psw[:, 0:512], lhsT=warm[0:64, 0:128],
                                 rhs=warm[0:64, 0:512], start=True, stop=True)

            # compute + output ----------------------------------------------------
            done = {"A": 0, "B": 0, "C": 0}

            def pick(n):
                tot = sum(done.values()) + n
                return max("ABC", key=lambda p: SHARE[p] * tot - done[p])

            act_pending = []   # lagged ACT-ring output DMAs

            for bc in range(2):
                for si, (lo, hi) in enumerate(STAGES[bc]):
                    st = stage_pool.tile([128, (hi - lo) * D], BF16, tag="stage")
                    for (p0, n) in _runs(lo, hi):
                        i, j0 = PAIRS[p0]
                        h = p0 // HALF
                        po = 64 * h
                        fcol = (i - 9 * h) * B_SH   # field col in ft's half
                        colbase = (p0 - h * HALF) * D
                        blk, bcol = colbase // 4096, colbase % 4096
                        if n <= 8:
                            ps = psum_pool.tile([128, 8 * D], F32, tag="ps8",
                                                bufs=2)
                        else:
                            ps = psum_pool.tile([128, RUN * D], F32, tag="ps",
                                                bufs=3)
                        for k in range(0, n, 8):
                            nk = min(8, n - k)
                            nc.tensor.matmul(
                                ps[:, k * D:(k + nk) * D],
                                lhsT=ft[po:po + 64,
                                        fcol + bc * 128:
                                        fcol + bc * 128 + 128],
                                rhs=w[blk][po:po + 64,
                                           bcol + k * D: bcol + (k + nk) * D],
                                start=True, stop=True,
                            )
                        st_sl = st[:, (p0 - lo) * D: (p0 - lo + n) * D]
                        fn_sl = fn[bc][:, j0 * D: (j0 + n) * D]
                        path = "A" if (bc, si) in FORCE_A else pick(n)
                        done[path] += n
                        if path == "A":
                            nc.vector.tensor_mul(st_sl, ps[:, 0:n * D], fn_sl)
                        else:
                            tmp = tmp_pool.tile([128, RUN * D], BF16, tag="tmp")
                            nc.scalar.copy(tmp[:, 0:n * D], ps[:, 0:n * D])
                            if path == "B":
                                nc.vector.tensor_mul(st_sl, tmp[:, 0:n * D],
                                                     fn_sl)
                            else:
                                nc.gpsimd.tensor_mul(st_sl, tmp[:, 0:n * D],
                                                     fn_sl)
                    if _out_ring(bc, si) == "sp":
                        nc.sync.dma_start(outs[(bc, si)][:, :], st[:, :])
                    else:
                        act_pending.append((outs[(bc, si)][:, :], st[:, :]))
                    if len(act_pending) > 1:
                        dst, src = act_pending.pop(0)
                        nc.scalar.dma_start(dst, src)
            for dst, src in act_pending:
                nc.scalar.dma_start(dst, src)
    nc.compile()
    return nc


def _pack_inputs(feature_emb, W):
    import ml_dtypes

    BF = ml_dtypes.bfloat16
    feature_emb = np.ascontiguousarray(feature_emb, dtype=np.float32)
    W = np.ascontiguousarray(W, dtype=np.float32)
    Wt = W.transpose(0, 2, 1)                      # [P, d, e]
    wpack = np.zeros((128, 4 * 4096), dtype=BF)
    wpack[0:64, :] = Wt[0:HALF].transpose(1, 0, 2).reshape(64, HALF * D).astype(BF)
    wpack[64:128, 0:(P - HALF) * D] = (
        Wt[HALF:P].transpose(1, 0, 2).reshape(64, (P - HALF) * D).astype(BF))
    in_maps = []
    for c in range(N_CORES):
        shard = feature_emb[c * B_SH:(c + 1) * B_SH]         # [256, 32, 64]
        # [d, f, b] per-field transposed features
        ftT = shard.transpose(2, 1, 0).astype(BF)            # [64, 32, 256]
        featT = np.zeros((128, 22 * B_SH), dtype=BF)
        # partitions 0:64 <- fields 0..9 (first-fields of pairs 0..255)
        featT[0:64, 0:10 * B_SH] = ftT[:, 0:10].reshape(64, 10 * B_SH)
        # partitions 64:128 <- fields 9..30 (first-fields of pairs 256..495)
        featT[64:128, :] = ftT[:, 9:31].reshape(64, 22 * B_SH)
        fnb = shard.reshape(B_SH, F * D).astype(BF)          # [256, 2048]
        srcs = {"ft": featT, "fn0": fnb[0:128], "fn1": fnb[128:256],
                **{f"w{i}": wpack[:, i * 4096:(i + 1) * 4096]
                   for i in range(4)}}
        in_maps.append({
            name: np.ascontiguousarray(srcs[dest][0:ph, c0:c1])
            for name, ring, dest, ph, c0, c1 in IN_CHUNKS
        })
    return in_maps


def kernel(feature_emb, W, _trace=False):
    from concourse.bass_utils import run_bass_kernel_spmd

    if "nc" not in _NC_CACHE:
        _NC_CACHE["nc"] = _build()
    nc = _NC_CACHE["nc"]
    in_maps = _pack_inputs(feature_emb, W)
    res = run_bass_kernel_spmd(nc, in_maps, core_ids=list(range(N_CORES)),
                               trace=_trace)
    out = np.empty((B, P * D), dtype=np.float32)
    for c in range(N_CORES):
        r = res.results[c]
        for bc in range(2):
            rows = slice(c * B_SH + bc * 128, c * B_SH + bc * 128 + 128)
            for si, (lo, hi) in enumerate(STAGES[bc]):
                out[rows, lo * D:hi * D] = r[f"o{bc}_{si}"].astype(np.float32)
    out = out.reshape(B, P, D)
    if _trace:
        return out, res
    return out


# revision 14
# speedup vs baseline: 1.0248x; 1.0242x over previous
"""Trainium2 Bass kernel for BilinearInteraction.

Reference math (B=2048, F=32 fields, D=64, P=496 field-pairs):
    for pair p=(i,j):  out[b,p,:] = (v_i @ W[p].T) * v_j
    v_i = feature_emb[:, i, :],  v_j = feature_emb[:, j, :]

Sharding: data-parallel over batch, 8 cores x 256 rows each; W replicated.
The fp32 output is 260MB (32.5MB/core) -> the kernel is output-write bound,
so the device writes bf16 (16.25MB/core) and the host upcasts; combined with
bf16 matmul operands the end-to-end relative error is ~3e-3, well inside the
2e-2 gate.

Per-core dataflow (all static, Tile-scheduled):
  - W is pre-transposed, cast to bf16 and packed host-side into
    wpack[128, 16384]: partitions 0:64 hold pairs 0..255 (cols p*64+e =
    W[p,e,d=partition]), partitions 64:128 hold pairs 256..495. Loaded as
    four resident [128,4096] tiles via column-sliced DMAs interleaved so
    the first compute stage's slice lands first.
  - featT[128, 5632] bf16 = per-field transposed features, the stationary
    matmul operand. A matmul requires lhsT/rhs to share a base partition,
    and pairs 0..255 (partitions 0:64) only ever use first-fields 0..9
    while pairs 256..495 (partitions 64:128) use 9..30 - so partitions
    0:64 hold fields 0..9 (col f*256+b) and partitions 64:128 hold fields
    9..30 (col (f-9)*256+b), one full-width DMA, no duplication.
  - featN[256, 2048] f32 = natural-layout features; the elementwise
    multiplier for consecutive pairs of one group is a contiguous slab.
  - Per (batch-half bc, output stage = 16..64 consecutive pairs; early
    stages are small so the output stream starts ~15us sooner): pairs
    grouped into "runs" (same first field, one 64-pair W block, <=16
    pairs). Each run: 1-2 matmuls [K=64,M=128]x[N<=512] into consecutive
    PSUM banks of one tile, then the PSUM x featN Hadamard product via
    one of two engine paths chosen to balance load (DVE TT from PSUM runs
    at ~95 elem/ns; GpSimd cannot read PSUM, so its path is ACT copy
    PSUM->SBUF f32 at ~95 then GpSimd TT at ~56; ACT/GpSimd are
    otherwise idle):
       path A (~65%): DVE  tensor_mul(psum_f32, featN_f32) -> stage bf16
       path C (~35%): ACT  copy psum -> tmp f32;
                      GPS  tensor_mul(tmp, featN_f32)      -> stage bf16
    Stage completes with one HWDGE DMA to the output row-block (the
    output lands directly in natural [b, p*64+e] layout). Early output
    DMAs ride the sync ring while inputs own the scalar ring; once the
    input stream drains, outputs alternate across both HWDGE rings.
"""

from itertools import combinations

import numpy as np

N_CORES = 8
B, F, D = 2048, 32, 64
P = 496
B_SH = B // N_CORES            # 256 batch rows per core
HALF = 256                     # pair index where the partition half flips
RUN = 16                       # max pairs per Hadamard op (2 PSUM banks)
GPS_FRAC = 0.35                # share of elements routed to the GpSimd path

# output stages as (pair_lo, pair_hi); first ones small to prime the pipe,
# last ones small to shorten the serial kernel tail
_BOUNDS0 = [0, 8, 16, 32, 64, 128, 192, 256, 320, 384, 448, 496]
_BOUNDS1 = [0, 64, 128, 192, 256, 320, 384, 448, 480, 496]
STAGES = {0: list(zip(_BOUNDS0[:-1], _BOUNDS0[1:])),
          1: list(zip(_BOUNDS1[:-1], _BOUNDS1[1:]))}

PAIRS = list(combinations(range(F), 2))

_NC_CACHE = {}


def _runs(lo, hi):
    """Runs of consecutive same-group pairs (<=RUN) in [lo,hi), not
    crossing 64-pair W-block boundaries."""
    runs = []
    p = lo
    while p < hi:
        i = PAIRS[p][0]
        e = p
        while (e + 1 < hi and PAIRS[e + 1][0] == i and (e + 1 - p) < RUN
               and (e + 1) % 64 != 0):
            e += 1
        runs.append((p, e - p + 1))
        p = e + 1
    return runs


def _build():
    import concourse.tile as tile
    from concourse import bacc, mybir

    F32 = mybir.dt.float32
    BF16 = mybir.dt.bfloat16
    nc = bacc.Bacc("TRN2", target_bir_lowering=False, debug=False,
                   enable_asserts=False, num_devices=N_CORES)

    wpack = nc.dram_tensor("wpack", [128, 4 * 4096], BF16, kind="ExternalInput").ap()
    featT = nc.dram_tensor("featT", [128, 22 * B_SH], BF16, kind="ExternalInput").ap()
    featN = nc.dram_tensor("featN", [B_SH, F * D], BF16, kind="ExternalInput").ap()
    out = nc.dram_tensor("out", [B_SH, P * D], BF16, kind="ExternalOutput").ap()

    with tile.TileContext(nc) as tc:
        with (
            tc.tile_pool(name="win", bufs=1) as win,
            tc.tile_pool(name="feat", bufs=1) as feat,
            tc.tile_pool(name="stage", bufs=8) as stage_pool,
            tc.tile_pool(name="tmp", bufs=8) as tmp_pool,
            tc.tile_pool(name="psum", bufs=4, space="PSUM") as psum_pool,
        ):
            # resident input tiles ------------------------------------------------
            w = [win.tile([128, 4096], BF16, name=f"w{blk}", tag=f"w{blk}")
                 for blk in range(4)]
            ft = feat.tile([128, 22 * B_SH], BF16, name="ft", tag="ft")
            fn = [feat.tile([128, F * D], BF16, name=f"fn{bc}", tag=f"fn{bc}")
                  for bc in range(2)]

            # issue order = first-compute order; fine slices first so the
            # pipeline primes fast (all on the scalar HWDGE ring; outputs
            # use the sync ring)
            # fields 0..9 of half 0 / 9..18 of half 1 + first W block first:
            # few, long-row DMAs (descriptor efficiency) in demand order
            # fn0 first: the first stages' multiplies gate PSUM recycling,
            # so their featN slab must land before PE fills the banks
            nc.scalar.dma_start(fn[0][:, :], featN[0:128, :])
            nc.scalar.dma_start(ft[:, 0:2560], featT[:, 0:2560])
            nc.scalar.dma_start(w[0][:, :], wpack[:, 0:4096])
            nc.scalar.dma_start(w[1][:, :], wpack[:, 4096:8192])
            nc.scalar.dma_start(ft[:, 2560:22 * B_SH], featT[:, 2560:22 * B_SH])
            nc.scalar.dma_start(w[2][:, :], wpack[:, 8192:12288])
            nc.scalar.dma_start(w[3][:, :], wpack[:, 12288:16384])
            nc.scalar.dma_start(fn[1][:, :], featN[128:256, :])

            # compute + output ----------------------------------------------------
            el_tot = el_gps = 0
            out_ring = [0]

            def out_dma(dst, src, bc, si):
                # inputs own the scalar ring for roughly the first half of
                # bc=0; after that alternate output DMAs across both HWDGE
                # rings so the output stream drains on 2 queues
                if bc == 0 and si < 5:
                    nc.sync.dma_start(dst, src)
                else:
                    eng = nc.sync if out_ring[0] % 2 == 0 else nc.scalar
                    out_ring[0] += 1
                    eng.dma_start(dst, src)

            for bc in range(2):
                stages = STAGES[bc]
                for si, (lo, hi) in enumerate(stages):
                    runs = _runs(lo, hi)
                    st = stage_pool.tile([128, (hi - lo) * D], BF16, tag="stage")
                    for ri, (p0, n) in enumerate(runs):
                        i, j0 = PAIRS[p0]
                        h = p0 // HALF
                        po = 64 * h
                        fcol = (i - 9 * h) * B_SH   # field col in ft's half
                        colbase = (p0 - h * HALF) * D
                        blk, bcol = colbase // 4096, colbase % 4096
                        if n <= 8:
                            ps = psum_pool.tile([128, 8 * D], F32, tag="ps8",
                                                bufs=2)
                        else:
                            ps = psum_pool.tile([128, RUN * D], F32, tag="ps",
                                                bufs=3)
                        for k in range(0, n, 8):
                            nk = min(8, n - k)
                            nc.tensor.matmul(
                                ps[:, k * D:(k + nk) * D],
                                lhsT=ft[po:po + 64,
                                        fcol + bc * 128:
                                        fcol + bc * 128 + 128],
                                rhs=w[blk][po:po + 64,
                                           bcol + k * D: bcol + (k + nk) * D],
                                start=True, stop=True,
                            )
                        st_sl = st[:, (p0 - lo) * D: (p0 - lo + n) * D]
                        fn_sl = fn[bc][:, j0 * D: (j0 + n) * D]
                        el_tot += n
                        if el_gps < GPS_FRAC * el_tot:
                            el_gps += n
                            tmp = tmp_pool.tile([128, RUN * D], BF16, tag="tmp")
                            nc.scalar.copy(tmp[:, 0:n * D], ps[:, 0:n * D])
                            nc.gpsimd.tensor_mul(st_sl, tmp[:, 0:n * D], fn_sl)
                        else:
                            nc.vector.tensor_mul(st_sl, ps[:, 0:n * D], fn_sl)
                    out_dma(out[bc * 128: bc * 128 + 128, lo * D: hi * D],
                            st[:, :], bc, si)
    nc.compile()
    return nc


def _pack_inputs(feature_emb, W):
    import ml_dtypes

    BF = ml_dtypes.bfloat16
    feature_emb = np.ascontiguousarray(feature_emb, dtype=np.float32)
    W = np.ascontiguousarray(W, dtype=np.float32)
    Wt = W.transpose(0, 2, 1)                      # [P, d, e]
    wpack = np.zeros((128, 4 * 4096), dtype=BF)
    wpack[0:64, :] = Wt[0:HALF].transpose(1, 0, 2).reshape(64, HALF * D).astype(BF)
    wpack[64:128, 0:(P - HALF) * D] = (
        Wt[HALF:P].transpose(1, 0, 2).reshape(64, (P - HALF) * D).astype(BF))
    in_maps = []
    for c in range(N_CORES):
        shard = feature_emb[c * B_SH:(c + 1) * B_SH]         # [256, 32, 64]
        # [d, f, b] per-field transposed features
        ftT = shard.transpose(2, 1, 0).astype(BF)            # [64, 32, 256]
        featT = np.zeros((128, 22 * B_SH), dtype=BF)
        # partitions 0:64 <- fields 0..9 (first-fields of pairs 0..255)
        featT[0:64, 0:10 * B_SH] = ftT[:, 0:10].reshape(64, 10 * B_SH)
        # partitions 64:128 <- fields 9..30 (first-fields of pairs 256..495)
        featT[64:128, :] = ftT[:, 9:31].reshape(64, 22 * B_SH)
        in_maps.append({
            "wpack": wpack,
            "featT": featT,
            "featN": shard.reshape(B_SH, F * D).astype(BF),
        })
    return in_maps


def kernel(feature_emb, W, _trace=False):
    from concourse.bass_utils import run_bass_kernel_spmd

    if "nc" not in _NC_CACHE:
        _NC_CACHE["nc"] = _build()
    nc = _NC_CACHE["nc"]
    in_maps = _pack_inputs(feature_emb, W)
    res = run_bass_kernel_spmd(nc, in_maps, core_ids=list(range(N_CORES)),
                               trace=_trace)
    full = np.concatenate(
        [res.results[c]["out"].astype(np.float32) for c in range(N_CORES)], axis=0)
    out = full.reshape(B, P, D)
    if _trace:
        return out, res
    return out



# revision 15
# speedup vs baseline: 1.1340x; 1.1065x over previous
"""Trainium2 Bass kernel for BilinearInteraction.

Reference math (B=2048, F=32 fields, D=64, P=496 field-pairs):
    for pair p=(i,j):  out[b,p,:] = (v_i @ W[p].T) * v_j
    v_i = feature_emb[:, i, :],  v_j = feature_emb[:, j, :]

Sharding: data-parallel over batch, 8 cores x 256 rows each; W replicated.
The fp32 output is 260MB (32.5MB/core) -> the kernel is output-write bound,
so the device writes bf16 (16.25MB/core) and the host upcasts; combined with
bf16 matmul operands the end-to-end relative error is ~3e-3, well inside the
2e-2 gate.

Per-core dataflow (all static, Tile-scheduled):
  - W is pre-transposed, cast to bf16 and packed host-side into
    wpack[128, 16384]: partitions 0:64 hold pairs 0..255 (cols p*64+e =
    W[p,e,d=partition]), partitions 64:128 hold pairs 256..495. Loaded as
    four resident [128,4096] tiles via column-sliced DMAs interleaved so
    the first compute stage's slice lands first.
  - featT[128, 5632] bf16 = per-field transposed features, the stationary
    matmul operand. A matmul requires lhsT/rhs to share a base partition,
    and pairs 0..255 (partitions 0:64) only ever use first-fields 0..9
    while pairs 256..495 (partitions 64:128) use 9..30 - so partitions
    0:64 hold fields 0..9 (col f*256+b) and partitions 64:128 hold fields
    9..30 (col (f-9)*256+b), one full-width DMA, no duplication.
  - featN[256, 2048] f32 = natural-layout features; the elementwise
    multiplier for consecutive pairs of one group is a contiguous slab.
  - Per (batch-half bc, output stage = 16..64 consecutive pairs; early
    stages are small so the output stream starts ~15us sooner): pairs
    grouped into "runs" (same first field, one 64-pair W block, <=16
    pairs). Each run: 1-2 matmuls [K=64,M=128]x[N<=512] into consecutive
    PSUM banks of one tile, then the PSUM x featN Hadamard product via
    one of two engine paths chosen to balance load (DVE TT from PSUM runs
    at ~95 elem/ns; GpSimd cannot read PSUM, so its path is ACT copy
    PSUM->SBUF f32 at ~95 then GpSimd TT at ~56; ACT/GpSimd are
    otherwise idle):
       path A (~65%): DVE  tensor_mul(psum_f32, featN_f32) -> stage bf16
       path C (~35%): ACT  copy psum -> tmp f32;
                      GPS  tensor_mul(tmp, featN_f32)      -> stage bf16
    Stage completes with one HWDGE DMA to the output row-block (the
    output lands directly in natural [b, p*64+e] layout). Early output
    DMAs ride the sync ring while inputs own the scalar ring; once the
    input stream drains, outputs alternate across both HWDGE rings.
"""

from itertools import combinations

import numpy as np

N_CORES = 8
B, F, D = 2048, 32, 64
P = 496
B_SH = B // N_CORES            # 256 batch rows per core
HALF = 256                     # pair index where the partition half flips
RUN = 16                       # max pairs per Hadamard op (2 PSUM banks)
GPS_FRAC = 0.35                # share of elements routed to the GpSimd path

# output stages as (pair_lo, pair_hi); first ones small to prime the pipe,
# last ones small to shorten the serial kernel tail
_BOUNDS0 = [0, 8, 16, 32, 64, 128, 192, 256, 320, 384, 448, 496]
_BOUNDS1 = [0, 64, 128, 192, 256, 320, 384, 448, 480, 496]
STAGES = {0: list(zip(_BOUNDS0[:-1], _BOUNDS0[1:])),
          1: list(zip(_BOUNDS1[:-1], _BOUNDS1[1:]))}

PAIRS = list(combinations(range(F), 2))

_NC_CACHE = {}


def _runs(lo, hi):
    """Runs of consecutive same-group pairs (<=RUN) in [lo,hi), not
    crossing 64-pair W-block boundaries."""
    runs = []
    p = lo
    while p < hi:
        i = PAIRS[p][0]
        e = p
        while (e + 1 < hi and PAIRS[e + 1][0] == i and (e + 1 - p) < RUN
               and (e + 1) % 64 != 0):
            e += 1
        runs.append((p, e - p + 1))
        p = e + 1
    return runs


def _build():
    import concourse.tile as tile
    from concourse import bacc, mybir

    F32 = mybir.dt.float32
    BF16 = mybir.dt.bfloat16
    nc = bacc.Bacc("TRN2", target_bir_lowering=False, debug=False,
                   enable_asserts=False, num_devices=N_CORES)

    wpack = nc.dram_tensor("wpack", [128, 4 * 4096], BF16, kind="ExternalInput").ap()
    featT = nc.dram_tensor("featT", [128, 22 * B_SH], BF16, kind="ExternalInput").ap()
    featN = nc.dram_tensor("featN", [B_SH, F * D], F32, kind="ExternalInput").ap()
    out = nc.dram_tensor("out", [B_SH, P * D], BF16, kind="ExternalOutput").ap()

    with tile.TileContext(nc) as tc:
        with (
            tc.tile_pool(name="win", bufs=1) as win,
            tc.tile_pool(name="feat", bufs=1) as feat,
            tc.tile_pool(name="stage", bufs=8) as stage_pool,
            tc.tile_pool(name="tmp", bufs=8) as tmp_pool,
            tc.tile_pool(name="psum", bufs=4, space="PSUM") as psum_pool,
        ):
            # resident input tiles ------------------------------------------------
            w = [win.tile([128, 4096], BF16, name=f"w{blk}", tag=f"w{blk}")
                 for blk in range(4)]
            ft = feat.tile([128, 22 * B_SH], BF16, name="ft", tag="ft")
            fn = [feat.tile([128, F * D], F32, name=f"fn{bc}", tag=f"fn{bc}")
                  for bc in range(2)]

            # issue order = first-compute order; fine slices first so the
            # pipeline primes fast (all on the scalar HWDGE ring; outputs
            # use the sync ring)
            # fields 0..9 of half 0 / 9..18 of half 1 + first W block first:
            # few, long-row DMAs (descriptor efficiency) in demand order
            # fn0 first: the first stages' multiplies gate PSUM recycling,
            # so their featN slab must land before PE fills the banks
            nc.scalar.dma_start(fn[0][:, :], featN[0:128, :])
            nc.scalar.dma_start(ft[:, 0:2560], featT[:, 0:2560])
            nc.scalar.dma_start(w[0][:, :], wpack[:, 0:4096])
            nc.scalar.dma_start(w[1][:, :], wpack[:, 4096:8192])
            nc.scalar.dma_start(ft[:, 2560:22 * B_SH], featT[:, 2560:22 * B_SH])
            nc.scalar.dma_start(w[2][:, :], wpack[:, 8192:12288])
            nc.scalar.dma_start(w[3][:, :], wpack[:, 12288:16384])
            nc.scalar.dma_start(fn[1][:, :], featN[128:256, :])

            # compute + output ----------------------------------------------------
            el_tot = el_gps = 0
            out_ring = [0]

            def out_dma(dst, src, bc, si):
                # inputs own the scalar ring for roughly the first half of
                # bc=0; after that alternate output DMAs across both HWDGE
                # rings so the output stream drains on 2 queues
                if bc == 0 and si < 8:
                    nc.sync.dma_start(dst, src)
                else:
                    eng = nc.sync if out_ring[0] % 2 == 0 else nc.scalar
                    out_ring[0] += 1
                    eng.dma_start(dst, src)

            for bc in range(2):
                stages = STAGES[bc]
                for si, (lo, hi) in enumerate(stages):
                    runs = _runs(lo, hi)
                    st = stage_pool.tile([128, (hi - lo) * D], BF16, tag="stage")
                    for ri, (p0, n) in enumerate(runs):
                        i, j0 = PAIRS[p0]
                        h = p0 // HALF
                        po = 64 * h
                        fcol = (i - 9 * h) * B_SH   # field col in ft's half
                        colbase = (p0 - h * HALF) * D
                        blk, bcol = colbase // 4096, colbase % 4096
                        if n <= 8:
                            ps = psum_pool.tile([128, 8 * D], F32, tag="ps8",
                                                bufs=2)
                        else:
                            ps = psum_pool.tile([128, RUN * D], F32, tag="ps",
                                                bufs=3)
                        for k in range(0, n, 8):
                            nk = min(8, n - k)
                            nc.tensor.matmul(
                                ps[:, k * D:(k + nk) * D],
                                lhsT=ft[po:po + 64,
                                        fcol + bc * 128:
                                        fcol + bc * 128 + 128],
                                rhs=w[blk][po:po + 64,
                                           bcol + k * D: bcol + (k + nk) * D],
                                start=True, stop=True,
                            )
                        st_sl = st[:, (p0 - lo) * D: (p0 - lo + n) * D]
                        fn_sl = fn[bc][:, j0 * D: (j0 + n) * D]
                        el_tot += n
                        if el_gps < GPS_FRAC * el_tot:
                            el_gps += n
                            tmp = tmp_pool.tile([128, RUN * D], F32, tag="tmp")
                            nc.scalar.copy(tmp[:, 0:n * D], ps[:, 0:n * D])
                            nc.gpsimd.tensor_mul(st_sl, tmp[:, 0:n * D], fn_sl)
                        else:
                            nc.vector.tensor_mul(st_sl, ps[:, 0:n * D], fn_sl)
                    out_dma(out[bc * 128: bc * 128 + 128, lo * D: hi * D],
                            st[:, :], bc, si)
    nc.compile()
    return nc


def _pack_inputs(feature_emb, W):
    import ml_dtypes

    BF = ml_dtypes.bfloat16
    feature_emb = np.ascontiguousarray(feature_emb, dtype=np.float32)
    W = np.ascontiguousarray(W, dtype=np.float32)
    Wt = W.transpose(0, 2, 1)                      # [P, d, e]
    wpack = np.zeros((128, 4 * 4096), dtype=BF)
    wpack[0:64, :] = Wt[0:HALF].transpose(1, 0, 2).reshape(64, HALF * D).astype(BF)
    wpack[64:128, 0:(P - HALF) * D] = (
        Wt[HALF:P].transpose(1, 0, 2).reshape(64, (P - HALF) * D).astype(BF))
    in_maps = []
    for c in range(N_CORES):
        shard = feature_emb[c * B_SH:(c + 1) * B_SH]         # [256, 32, 64]
        # [d, f, b] per-field transposed features
        ftT = shard.transpose(2, 1, 0).astype(BF)            # [64, 32, 256]
        featT = np.zeros((128, 22 * B_SH), dtype=BF)
        # partitions 0:64 <- fields 0..9 (first-fields of pairs 0..255)
        featT[0:64, 0:10 * B_SH] = ftT[:, 0:10].reshape(64, 10 * B_SH)
        # partitions 64:128 <- fields 9..30 (first-fields of pairs 256..495)
        featT[64:128, :] = ftT[:, 9:31].reshape(64, 22 * B_SH)
        in_maps.append({
            "wpack": wpack,
            "featT": featT,
            "featN": np.ascontiguousarray(shard.reshape(B_SH, F * D)),
        })
    return in_maps


def kernel(feature_emb, W, _trace=False):
    from concourse.bass_utils import run_bass_kernel_spmd

    if "nc" not in _NC_CACHE:
        _NC_CACHE["nc"] = _build()
    nc = _NC_CACHE["nc"]
    in_maps = _pack_inputs(feature_emb, W)
    res = run_bass_kernel_spmd(nc, in_maps, core_ids=list(range(N_CORES)),
                               trace=_trace)
    full = np.concatenate(
        [res.results[c]["out"].astype(np.float32) for c in range(N_CORES)], axis=0)
    out = full.reshape(B, P, D)
    if _trace:
        return out, res
    return out



# revision 18
# speedup vs baseline: 1.1761x; 1.0372x over previous
"""Trainium2 Bass kernel for BilinearInteraction.

Reference math (B=2048, F=32 fields, D=64, P=496 field-pairs):
    for pair p=(i,j):  out[b,p,:] = (v_i @ W[p].T) * v_j
    v_i = feature_emb[:, i, :],  v_j = feature_emb[:, j, :]

Sharding: data-parallel over batch, 8 cores x 256 rows each; W replicated.
The fp32 output is 260MB (32.5MB/core) -> the kernel is output-write bound,
so the device writes bf16 (16.25MB/core) and the host upcasts; combined with
bf16 matmul operands the end-to-end relative error is ~3e-3, well inside the
2e-2 gate.

Per-core dataflow (all static, Tile-scheduled):
  - W is pre-transposed, cast to bf16 and packed host-side into
    wpack[128, 16384]: partitions 0:64 hold pairs 0..255 (cols p*64+e =
    W[p,e,d=partition]), partitions 64:128 hold pairs 256..495. Loaded as
    four resident [128,4096] tiles via column-sliced DMAs interleaved so
    the first compute stage's slice lands first.
  - featT[128, 5632] bf16 = per-field transposed features, the stationary
    matmul operand. A matmul requires lhsT/rhs to share a base partition,
    and pairs 0..255 (partitions 0:64) only ever use first-fields 0..9
    while pairs 256..495 (partitions 64:128) use 9..30 - so partitions
    0:64 hold fields 0..9 (col f*256+b) and partitions 64:128 hold fields
    9..30 (col (f-9)*256+b), one full-width DMA, no duplication.
  - featN[256, 2048] f32 = natural-layout features; the elementwise
    multiplier for consecutive pairs of one group is a contiguous slab.
  - Per (batch-half bc, output stage = 16..64 consecutive pairs; early
    stages are small so the output stream starts ~15us sooner): pairs
    grouped into "runs" (same first field, one 64-pair W block, <=16
    pairs). Each run: 1-2 matmuls [K=64,M=128]x[N<=512] into consecutive
    PSUM banks of one tile, then the PSUM x featN Hadamard product via
    one of two engine paths chosen to balance load (DVE TT from PSUM runs
    at ~95 elem/ns; GpSimd cannot read PSUM, so its path is ACT copy
    PSUM->SBUF f32 at ~95 then GpSimd TT at ~56; ACT/GpSimd are
    otherwise idle):
       path A (~65%): DVE  tensor_mul(psum_f32, featN_f32) -> stage bf16
       path C (~35%): ACT  copy psum -> tmp f32;
                      GPS  tensor_mul(tmp, featN_f32)      -> stage bf16
    Stage completes with one HWDGE DMA to the output row-block (the
    output lands directly in natural [b, p*64+e] layout). Early output
    DMAs ride the sync ring while inputs own the scalar ring; once the
    input stream drains, outputs alternate across both HWDGE rings.
"""

from itertools import combinations

import numpy as np

N_CORES = 8
B, F, D = 2048, 32, 64
P = 496
B_SH = B // N_CORES            # 256 batch rows per core
HALF = 256                     # pair index where the partition half flips
RUN = 16                       # max pairs per Hadamard op (2 PSUM banks)
GPS_FRAC = 0.35                # share of elements routed to the GpSimd path

# output stages as (pair_lo, pair_hi); first ones small to prime the pipe,
# last ones small to shorten the serial kernel tail
_BOUNDS0 = [0, 8, 16, 32, 64, 128, 192, 256, 320, 384, 448, 496]
_BOUNDS1 = [0, 64, 128, 192, 256, 320, 384, 448, 480, 496]
STAGES = {0: list(zip(_BOUNDS0[:-1], _BOUNDS0[1:])),
          1: list(zip(_BOUNDS1[:-1], _BOUNDS1[1:]))}

PAIRS = list(combinations(range(F), 2))

_NC_CACHE = {}


def _runs(lo, hi):
    """Runs of consecutive same-group pairs (<=RUN) in [lo,hi), not
    crossing 64-pair W-block boundaries."""
    runs = []
    p = lo
    while p < hi:
        i = PAIRS[p][0]
        e = p
        while (e + 1 < hi and PAIRS[e + 1][0] == i and (e + 1 - p) < RUN
               and (e + 1) % 64 != 0):
            e += 1
        runs.append((p, e - p + 1))
        p = e + 1
    return runs


def _build():
    import concourse.tile as tile
    from concourse import bacc, mybir

    F32 = mybir.dt.float32
    BF16 = mybir.dt.bfloat16
    nc = bacc.Bacc("TRN2", target_bir_lowering=False, debug=False,
                   enable_asserts=False, num_devices=N_CORES)

    wpack = nc.dram_tensor("wpack", [128, 4 * 4096], BF16, kind="ExternalInput").ap()
    featT = nc.dram_tensor("featT", [128, 22 * B_SH], BF16, kind="ExternalInput").ap()
    featN = nc.dram_tensor("featN", [B_SH, F * D], F32, kind="ExternalInput").ap()
    out = nc.dram_tensor("out", [B_SH, P * D], BF16, kind="ExternalOutput").ap()

    with tile.TileContext(nc) as tc:
        with (
            tc.tile_pool(name="win", bufs=1) as win,
            tc.tile_pool(name="feat", bufs=1) as feat,
            tc.tile_pool(name="stage", bufs=8) as stage_pool,
            tc.tile_pool(name="tmp", bufs=8) as tmp_pool,
            tc.tile_pool(name="psum", bufs=4, space="PSUM") as psum_pool,
        ):
            # resident input tiles ------------------------------------------------
            w = [win.tile([128, 4096], BF16, name=f"w{blk}", tag=f"w{blk}")
                 for blk in range(4)]
            ft = feat.tile([128, 22 * B_SH], BF16, name="ft", tag="ft")
            fn = [feat.tile([128, F * D], F32, name=f"fn{bc}", tag=f"fn{bc}")
                  for bc in range(2)]

            # issue order = first-compute order; fine slices first so the
            # pipeline primes fast (all on the scalar HWDGE ring; outputs
            # use the sync ring)
            # fields 0..9 of half 0 / 9..18 of half 1 + first W block first:
            # few, long-row DMAs (descriptor efficiency) in demand order
            # fn0 first: the first stages' multiplies gate PSUM recycling,
            # so their featN slab must land before PE fills the banks
            nc.scalar.dma_start(fn[0][:, :], featN[0:128, :])
            nc.scalar.dma_start(ft[:, 0:2560], featT[:, 0:2560])
            # w0 split so the first stages' matmuls start ~1us earlier
            nc.scalar.dma_start(w[0][:, 0:1024], wpack[:, 0:1024])
            nc.scalar.dma_start(w[0][:, 1024:4096], wpack[:, 1024:4096])
            nc.scalar.dma_start(w[1][:, :], wpack[:, 4096:8192])
            nc.scalar.dma_start(ft[:, 2560:22 * B_SH], featT[:, 2560:22 * B_SH])
            nc.scalar.dma_start(w[2][:, :], wpack[:, 8192:12288])
            nc.scalar.dma_start(w[3][:, :], wpack[:, 12288:16384])
            nc.scalar.dma_start(fn[1][:, :], featN[128:256, :])

            # compute + output ----------------------------------------------------
            el_tot = el_gps = 0
            out_ring = [0]

            def out_dma(dst, src, bc, si):
                # inputs own the scalar ring for roughly the first half of
                # bc=0; after that alternate output DMAs across both HWDGE
                # rings so the output stream drains on 2 queues
                if bc == 0 and si < 5:
                    nc.sync.dma_start(dst, src)
                else:
                    eng = nc.sync if out_ring[0] % 2 == 0 else nc.scalar
                    out_ring[0] += 1
                    eng.dma_start(dst, src)

            for bc in range(2):
                stages = STAGES[bc]
                for si, (lo, hi) in enumerate(stages):
                    runs = _runs(lo, hi)
                    st = stage_pool.tile([128, (hi - lo) * D], BF16, tag="stage")
                    for ri, (p0, n) in enumerate(runs):
                        i, j0 = PAIRS[p0]
                        h = p0 // HALF
                        po = 64 * h
                        fcol = (i - 9 * h) * B_SH   # field col in ft's half
                        colbase = (p0 - h * HALF) * D
                        blk, bcol = colbase // 4096, colbase % 4096
                        if n <= 8:
                            ps = psum_pool.tile([128, 8 * D], F32, tag="ps8",
                                                bufs=2)
                        else:
                            ps = psum_pool.tile([128, RUN * D], F32, tag="ps",
                                                bufs=3)
                        for k in range(0, n, 8):
                            nk = min(8, n - k)
                            nc.tensor.matmul(
                                ps[:, k * D:(k + nk) * D],
                                lhsT=ft[po:po + 64,
                                        fcol + bc * 128:
                                        fcol + bc * 128 + 128],
                                rhs=w[blk][po:po + 64,
                                           bcol + k * D: bcol + (k + nk) * D],
                                start=True, stop=True,
                            )
                        st_sl = st[:, (p0 - lo) * D: (p0 - lo + n) * D]
                        fn_sl = fn[bc][:, j0 * D: (j0 + n) * D]
                        el_tot += n
                        # tail stages stay on the short DVE chain
                        if (el_gps < GPS_FRAC * el_tot
                                and not (bc == 1 and si >= 7)):
                            el_gps += n
                            tmp = tmp_pool.tile([128, RUN * D], F32, tag="tmp")
                            nc.scalar.copy(tmp[:, 0:n * D], ps[:, 0:n * D])
                            nc.gpsimd.tensor_mul(st_sl, tmp[:, 0:n * D], fn_sl)
                        else:
                            nc.vector.tensor_mul(st_sl, ps[:, 0:n * D], fn_sl)
                    out_dma(out[bc * 128: bc * 128 + 128, lo * D: hi * D],
                            st[:, :], bc, si)
    nc.compile()
    return nc


def _pack_inputs(feature_emb, W):
    import ml_dtypes

    BF = ml_dtypes.bfloat16
    feature_emb = np.ascontiguousarray(feature_emb, dtype=np.float32)
    W = np.ascontiguousarray(W, dtype=np.float32)
    Wt = W.transpose(0, 2, 1)                      # [P, d, e]
    wpack = np.zeros((128, 4 * 4096), dtype=BF)
    wpack[0:64, :] = Wt[0:HALF].transpose(1, 0, 2).reshape(64, HALF * D).astype(BF)
    wpack[64:128, 0:(P - HALF) * D] = (
        Wt[HALF:P].transpose(1, 0, 2).reshape(64, (P - HALF) * D).astype(BF))
    in_maps = []
    for c in range(N_CORES):
        shard = feature_emb[c * B_SH:(c + 1) * B_SH]         # [256, 32, 64]
        # [d, f, b] per-field transposed features
        ftT = shard.transpose(2, 1, 0).astype(BF)            # [64, 32, 256]
        featT = np.zeros((128, 22 * B_SH), dtype=BF)
        # partitions 0:64 <- fields 0..9 (first-fields of pairs 0..255)
        featT[0:64, 0:10 * B_SH] = ftT[:, 0:10].reshape(64, 10 * B_SH)
        # partitions 64:128 <- fields 9..30 (first-fields of pairs 256..495)
        featT[64:128, :] = ftT[:, 9:31].reshape(64, 22 * B_SH)
        in_maps.append({
            "wpack": wpack,
            "featT": featT,
            "featN": np.ascontiguousarray(shard.reshape(B_SH, F * D)),
        })
    return in_maps


def kernel(feature_emb, W, _trace=False):
    from concourse.bass_utils import run_bass_kernel_spmd

    if "nc" not in _NC_CACHE:
        _NC_CACHE["nc"] = _build()
    nc = _NC_CACHE["nc"]
    in_maps = _pack_inputs(feature_emb, W)
    res = run_bass_kernel_spmd(nc, in_maps, core_ids=list(range(N_CORES)),
                               trace=_trace)
    full = np.concatenate(
        [res.results[c]["out"].astype(np.float32) for c in range(N_CORES)], axis=0)
    out = full.reshape(B, P, D)
    if _trace:
        return out, res
    return out

